# revision 26
# baseline (speedup 1.0000x reference)
"""3-layer GCN (BaseNet) on 8 Trainium2 NeuronCores.

Strategy (self-contained; hardcoded for N=100000, E=1.6M, D=128, LAT=32x3, G=1000):
 - Nodes sharded contiguously across 8 cores (12500/core); each core owns all
   edges whose dst lies in its shard.
 - Per core, shard nodes are sorted by in-degree so the edge aggregation
   becomes "rounds": round r gathers hs[src_r(n)] for the prefix of nodes with
   indeg > r, landing slot-aligned with the accumulator -> plain DVE adds,
   no scatter needed.
 - Gathers are 128-row indirect DMAs from a per-core HBM replica of the
   scaled feature table hs = deg^-1/2 * (h @ W); replicas built by AllGather.
 - Math identity used: agg + self = dis * (sum_e hs[src] + hs[own]); the
   edge coefficient dis[src]*dis[dst] factors into per-node pre/post scales.
 - Backward slicing: layer-2 output is only needed at readout nodes and their
   in-neighbors (~17K nodes); layer-3 only at the 1000 readout nodes. Cuts
   gather volume ~2.7x.
 - Readout (first node of each graph) + 2-layer MLP + log_softmax on device.

Perf notes (why it looks the way it does):
 - Per-dispatch PJRT arg staging costs ~90-130 us/MB/core, so all
   topology-derived tables (gather indices, degree grids, self/readout maps)
   are NEFF-baked inline constants; each core fetches its slice of the
   [8*128, W] stacked tables with one partition_id-offset indirect DMA.
   Only x (as fp8_e4m3, halving the dominant upload) and the model weights
   are true per-dispatch inputs.
 - Layer-1/2 feature tables (stripes + AllGather replicas) are fp8_e4m3:
   halves collective wire and gather HBM traffic; DVE adds consume fp8
   operands directly. Layer-3 stays bf16 (feeds the readout, error budget).
 - Layer-1 finish + layer-2 transform + stripe-2 writes are emitted per
   4-chunk group in reverse chunk order: low-indegree chunks complete after
   the first few gather rounds, so their tail work overlaps the remaining
   layer-1 gathers and AllGather-2 starts earlier.
 - Each indirect gather (128 rows) costs ~0.3-1 us fixed on the Pool engine
   (SWDGE); instruction count is the floor. Batched alternatives were probed
   on HW and are unusable here: multi-index offset APs mis-unroll in walrus,
   and dma_gather/ap_gather (custom GPSIMD ucode) fail neuronxcc codegen.
"""
import os
import sys

for _p in ("/opt/trn_rl_repo", "/root/.axon_site/_ro/trn_rl_repo"):
    if os.path.isdir(_p) and _p not in sys.path:
        sys.path.insert(0, _p)

import ml_dtypes
import numpy as np

import concourse.bass as bass
import concourse.mybir as mybir
import concourse.tile as tile_mod
from concourse.masks import make_identity
from concourse.vector_clock import ScopedClock

NC = 8
F = 32          # latent feature dim
P = 128

FP32 = mybir.dt.float32
I32 = mybir.dt.int32
AF = mybir.ActivationFunctionType
ALU = mybir.AluOpType

# ---------------------------------------------------------------------------
# walrus compat: this build rejects >1 sem wait per instruction. Spread the
# Tile drain's waits and any multi-wait instruction across nop carriers.
# ---------------------------------------------------------------------------

def _patched_drain_and_barrier(self, tick_clock, wait_clock):
    probe = self.nc.sync.nop(nofuse=True, hint="drain_wait_carrier")
    wait_clock.add_sem_waits(probe.ins, ScopedClock({None: tick_clock.global_clock}))
    si = probe.ins.sync_info
    waits = list(si.on_wait) if si is not None else []
    probe.ins.sync_info = mybir.SyncInfo(
        on_wait=waits[:1], on_update=list(si.on_update) if si is not None else []
    )
    for k in range(1, len(waits)):
        extra = self.nc.sync.nop(nofuse=True, hint=f"drain_wait_carrier_{k}")
        extra.ins.sync_info = mybir.SyncInfo(on_wait=waits[k : k + 1], on_update=[])
    self.nc.sync.drain()
    self.nc.all_engine_barrier()
    assert self.sems is not None
    popped = self.nc._tile_sem_poison_stack.pop()
    assert popped is self._sem_poison
    self.nc.clear_and_free_semaphores(list(self.sems.allocated().values()))
    self.nc.all_engine_barrier()


tile_mod.TileContext._drain_and_barrier = _patched_drain_and_barrier


def _split_waits(nc, max_waits=1):
    ctr = 0
    for fn in nc.m.functions:
        for bb in fn.blocks:
            out = []
            changed = False
            for inst in bb.instructions:
                si = inst.sync_info
                n = len(si.on_wait) if si is not None else 0
                if n > max_waits:
                    waits = list(si.on_wait)
                    keep = waits[-max_waits:]
                    extra = waits[:-max_waits]
                    for k in range(0, len(extra), max_waits):
                        nop = mybir.InstNoOp(
                            name=f"waitnop-{ctr}",
                            engine=inst.engine,
                            bass_nofuse=True,
                            sync_info=mybir.SyncInfo(
                                on_wait=extra[k : k + max_waits], on_update=[]
                            ),
                        )
                        ctr += 1
                        out.append(nop)
                    inst.sync_info = mybir.SyncInfo(
                        on_wait=keep, on_update=list(si.on_update)
                    )
                    changed = True
                out.append(inst)
            if changed:
                bb.instructions = out
    return ctr


# ---------------------------------------------------------------------------
# SPMD runner (PJRT/axon), build once / run many.
# ---------------------------------------------------------------------------

class _SpmdRunner:
    def __init__(self, nc, n_cores=NC):
        import jax
        from jax.sharding import Mesh, NamedSharding, PartitionSpec
        from jax.experimental.shard_map import shard_map
        from concourse.bass2jax import (
            _bass_exec_p,
            install_neuronx_cc_hook,
            partition_id_tensor,
        )

        self.jax = jax
        install_neuronx_cc_hook()
        self.n_cores = n_cores
        partition_name = nc.partition_id_tensor.name if nc.partition_id_tensor else None

        in_names, out_names, out_avals = [], [], []
        for alloc in nc.m.functions[0].allocations:
            if not isinstance(alloc, mybir.MemoryLocationSet):
                continue
            name = alloc.memorylocations[0].name
            if alloc.kind == "ExternalInput":
                if name != partition_name:
                    in_names.append(name)
            elif alloc.kind == "ExternalOutput":
                out_avals.append(
                    jax.core.ShapedArray(
                        tuple(alloc.tensor_shape), mybir.dt.np(alloc.dtype)
                    )
                )
                out_names.append(name)
        self.in_names, self.out_names, self.out_avals = in_names, out_names, out_avals
        n_params = len(in_names)
        all_in = in_names + out_names + ([partition_name] if partition_name else [])

        def _body(*args):
            operands = list(args)
            if partition_name is not None:
                operands.append(partition_id_tensor())
            return tuple(
                _bass_exec_p.bind(
                    *operands,
                    out_avals=tuple(out_avals),
                    in_names=tuple(all_in),
                    out_names=tuple(out_names),
                    lowering_input_output_aliases=(),
                    sim_require_finite=True,
                    sim_require_nnan=True,
                    nc=nc,
                )
            )

        devices = jax.devices()[:n_cores]
        assert len(devices) == n_cores, f"need {n_cores} cores, saw {len(jax.devices())}"
        mesh = Mesh(np.asarray(devices), ("core",))
        self.sharded = jax.jit(
            shard_map(
                _body,
                mesh=mesh,
                in_specs=(PartitionSpec("core"),) * (n_params + len(out_names)),
                out_specs=(PartitionSpec("core"),) * len(out_names),
                check_rep=False,
            ),
            keep_unused=True,
        )
        self.sharding = NamedSharding(mesh, PartitionSpec("core"))

    def stage(self, in_maps):
        args = []
        for name in self.in_names:
            cat = np.concatenate(
                [np.ascontiguousarray(in_maps[c][name]) for c in range(self.n_cores)],
                axis=0,
            )
            args.append(self.jax.device_put(cat, self.sharding))
        for av in self.out_avals:
            z = np.zeros((self.n_cores * av.shape[0], *av.shape[1:]), av.dtype)
            args.append(self.jax.device_put(z, self.sharding))
        return args

    def run_staged(self, args):
        outs = self.sharded(*args)
        self.jax.block_until_ready(outs)
        return outs

    def results(self, outs):
        n = self.n_cores
        return [
            {
                name: np.asarray(outs[i]).reshape(n, *self.out_avals[i].shape)[c]
                for i, name in enumerate(self.out_names)
            }
            for c in range(n)
        ]


# ---------------------------------------------------------------------------
# Host preprocessing
# ---------------------------------------------------------------------------

def _cdiv(a, b):
    return -(-a // b)


class _AggStruct:
    """Slot-aligned gather rounds for one layer, unified across cores."""

    def __init__(self, active, indeg, shard):
        N = active.shape[0]
        self.N = N
        self.shard = shard
        self.pos = np.full(N, -1, np.int64)
        self.perm = []
        counts = []
        for c in range(NC):
            nodes = np.flatnonzero(active[c * shard : (c + 1) * shard]) + c * shard
            order = np.argsort(-indeg[nodes], kind="stable")
            pc = nodes[order]
            self.perm.append(pc)
            self.pos[pc] = np.arange(len(pc))
            counts.append(len(pc))
        self.counts = counts
        self.chunks = max(1, _cdiv(max(counts), P))
        SPa = self.chunks * P
        self.SPa = SPa

        # rounds: cols per round, unified (max over cores)
        R = 0
        percore_d = []
        for c in range(NC):
            d = indeg[self.perm[c]]  # sorted desc
            R = max(R, int(d[0]) if len(d) else 0)
            percore_d.append(d)
        cols = []
        for r in range(R):
            m = 0
            for c in range(NC):
                nr = int(np.count_nonzero(percore_d[c] > r))
                m = max(m, _cdiv(nr, P))
            if m == 0:
                break
            cols.append(m)
        self.cols = cols
        self.NI = sum(cols)

    def build_idx(self, e_src, e_dst, t_row, Z):
        off = np.zeros(len(self.cols) + 1, np.int64)
        off[1:] = np.cumsum(self.cols)
        idx = np.full((NC, self.NI, P), Z, np.int32)
        if len(e_dst) and self.NI:
            key = (e_dst // self.shard) * self.SPa + self.pos[e_dst]
            order_e = np.argsort(key, kind="stable")
            sk = key[order_e]
            gs = np.r_[0, np.flatnonzero(np.diff(sk)) + 1]
            seq = np.arange(len(sk)) - np.repeat(gs, np.diff(np.r_[gs, len(sk)]))
            c_e = sk // self.SPa
            k_e = sk % self.SPa
            instr = off[seq] + (k_e // P)
            idx[c_e, instr, k_e % P] = t_row[e_src[order_e]].astype(np.int32)
        return idx  # [NC, NI, 128]

    def self_idx(self, t_row, Z):
        """[NC, chunks, 128] gather-own-row indices (for accumulator init)."""
        out = np.full((NC, self.chunks, P), Z, np.int32)
        for c in range(NC):
            k = np.arange(len(self.perm[c]))
            out[c, k // P, k % P] = t_row[self.perm[c]].astype(np.int32)
        return out

    def grid(self, vals, pad, dtype=np.float32):
        """[NC, 128, chunks] per-slot values in (p, ch) layout."""
        g = np.full((NC, P, self.chunks), pad, dtype)
        for c in range(NC):
            k = np.arange(len(self.perm[c]))
            g[c, k % P, k // P] = vals[self.perm[c]]
        return g

    def local_row(self, nodes):
        """flat stripe row (p*chunks + ch) of nodes within their own core."""
        pos = self.pos[nodes]
        return (pos % P) * self.chunks + pos // P

    def table_row(self, nodes):
        """global table row across core stripes."""
        c = np.asarray(nodes) // self.shard
        pos = self.pos[nodes]
        return c * self.SPa + (pos % P) * self.chunks + pos // P


def _preprocess(x, edge_index, batch, num_graphs):
    N = x.shape[0]
    E = edge_index.shape[1]
    G = int(num_graphs)
    shard = N // NC
    assert N % NC == 0
    src = edge_index[0].astype(np.int64)
    dst = edge_index[1].astype(np.int64)
    indeg = np.bincount(dst, minlength=N)
    deg = (indeg + 1).astype(np.float32)

    idxg = np.searchsorted(batch, np.arange(G, dtype=np.int64))

    # backward slicing
    act3 = np.zeros(N, bool)
    act3[idxg] = True
    m3 = act3[dst]
    act2 = np.zeros(N, bool)
    act2[src[m3]] = True
    act2[idxg] = True
    m2 = act2[dst]
    act1 = np.ones(N, bool)

    indeg2 = np.bincount(dst[m2], minlength=N)
    indeg3 = np.bincount(dst[m3], minlength=N)

    # layer 1: table covers all nodes, but only act1 nodes (h1 consumers:
    # sources of layer-2 edges + readout) aggregate. Sorting each shard as
    # [act1 by indeg desc, then inactive] keeps the accumulator == hs1 buffer
    # (self term free) while rounds cover only the active prefix.
    act1 = np.zeros(N, bool)
    act1[src[m2]] = True
    act1[idxg] = True
    m1 = act1[dst]
    indeg1m = np.where(act1, indeg, 0)
    a1 = _AggStruct(np.ones(N, bool), indeg1m, shard)
    Z1 = NC * a1.SPa
    t1 = a1.table_row(np.arange(N))
    idx1_raw = a1.build_idx(src[m1], dst[m1], t1, Z1)

    # layer 2: accumulate only at act2 nodes; table2 is full (t1 coords)
    a2 = _AggStruct(act2, indeg2, shard)
    idx2_raw = a2.build_idx(src[m2], dst[m2], t1, Z1)
    # layer 3: accumulate at readout nodes; table3 covers act2 (perm2 stripes)
    t3 = np.zeros(N, np.int64)
    a2n = np.flatnonzero(act2)
    t3[a2n] = a2.table_row(a2n)
    Z3 = NC * a2.SPa
    a3 = _AggStruct(act3, indeg3, shard)
    idx3_raw = a3.build_idx(src[m3], dst[m3], t3, Z3)

    # stripe-local self rows (gathered from the core's own stripe, pre-AllGather)
    # local stripe row of own node n = t(n) - c*SPa ; zero row = SPa (extra row)
    selfl2 = np.full((NC, a2.chunks, P), a1.SPa, np.int32)
    for c in range(NC):
        k = np.arange(len(a2.perm[c]))
        selfl2[c, k // P, k % P] = (t1[a2.perm[c]] - c * a1.SPa).astype(np.int32)
    selfl3 = np.full((NC, a3.chunks, P), a2.SPa, np.int32)
    for c in range(NC):
        k = np.arange(len(a3.perm[c]))
        selfl3[c, k // P, k % P] = (t3[a3.perm[c]] - c * a2.SPa).astype(np.int32)

    # grids
    deg1_g = a1.grid(deg, 1.0)
    deg2_g = a2.grid(deg, 1.0)
    deg3_g = a3.grid(deg, 1.0)

    # x transposed+permuted per core: [128, SPa1]
    xT = np.zeros((NC, P, a1.SPa), np.float32)
    for c in range(NC):
        xT[c, :, : len(a1.perm[c])] = x[a1.perm[c]].T

    # readout: slot p of core c = a3.perm[c][p]
    r1_idx = np.zeros((NC, P, 1), np.int32)
    r2_idx = np.zeros((NC, P, 1), np.int32)
    graph_of = -np.ones(N, np.int64)
    graph_of[idxg] = np.arange(G)
    slot_graphs = []
    for c in range(NC):
        nodes = a3.perm[c]
        assert len(nodes) <= P, f"core {c} has {len(nodes)} graphs (>128)"
        r1_idx[c, : len(nodes), 0] = a1.local_row(nodes)
        r2_idx[c, : len(nodes), 0] = a2.local_row(nodes)
        slot_graphs.append(graph_of[nodes])

    # gather idx arrays, transposed to [128, NI] for contiguous DMA
    def tr(a):  # [NC, NI, 128] -> [NC, 128, NI]
        return np.ascontiguousarray(np.transpose(a, (0, 2, 1)))

    idx1 = tr(idx1_raw)
    idx2 = tr(idx2_raw)
    idx3 = tr(idx3_raw)
    sl2 = tr(selfl2)
    sl3 = tr(selfl3)
    if idx1.shape[2] == 0:  # degenerate: no edges at all
        idx1 = np.full((NC, P, 1), Z1, np.int32)
        a1.cols = [1]
    if idx2.shape[2] == 0:
        idx2 = np.full((NC, P, 1), Z1, np.int32)
        a2.cols = [1]
    if idx3.shape[2] == 0:
        idx3 = np.full((NC, P, 1), Z3, np.int32)
        a3.cols = [1]

    meta = dict(
        CH1=a1.chunks, CH2=a2.chunks, CH3=a3.chunks,
        SP1=a1.SPa, SP2=a2.SPa, SP3=a3.SPa,
        cols1=a1.cols,
        cols2=a2.cols,
        cols3=a3.cols,
        NI1=idx1.shape[2], NI2=idx2.shape[2], NI3=idx3.shape[2],
        G=G,
    )
    per_core = [
        dict(
            xT=xT[c], deg1=deg1_g[c], deg2=deg2_g[c], deg3=deg3_g[c],
            idx1=idx1[c], idx2=idx2[c], idx3=idx3[c],
            sl2=sl2[c], sl3=sl3[c],
            r1=r1_idx[c], r2=r2_idx[c],
        )
        for c in range(NC)
    ]
    return meta, per_core, slot_graphs


# ---------------------------------------------------------------------------
# Device program
# ---------------------------------------------------------------------------

ABLATE = set()  # {"collectives", "gathers", "transforms"} for timing experiments


def _build(meta, consts):
    CH1, CH2, CH3 = meta["CH1"], meta["CH2"], meta["CH3"]
    SP1, SP2 = meta["SP1"], meta["SP2"]
    NI1, NI2, NI3 = meta["NI1"], meta["NI2"], meta["NI3"]
    T1_ROWS = NC * SP1 + P
    T3_ROWS = NC * SP2 + P
    BF16 = mybir.dt.bfloat16
    FP8 = mybir.dt.float8e4
    NIT = NI1 + NI2 + NI3
    CHT = CH1 + CH2 + CH3
    SLT = CH2 + CH3 + 2

    # 4x SWDGE descriptor ring (4096 descs = 32 indirect gathers in
    # flight vs default 8): the gather stream is completion-latency
    # bound through this ring; 64KB measured best (96KB regressed)
    nc = bass.Bass(dynamic_dma_scratch_size=65536, num_swdge_queues=2)
    dp = nc.declare_dram_parameter
    xT_e = dp("xT", [P, SP1], FP8, isOutput=False)
    W1_e = dp("W1", [P, F], FP32, isOutput=False)
    W2_e = dp("W2", [F, F], FP32, isOutput=False)
    W3_e = dp("W3", [F, F], FP32, isOutput=False)
    b1_e = dp("b1", [1, F], FP32, isOutput=False)
    b2_e = dp("b2", [1, F], FP32, isOutput=False)
    b3_e = dp("b3", [1, F], FP32, isOutput=False)
    l1w_e = dp("l1w", [96, P], FP32, isOutput=False)
    l1b_e = dp("l1b", [1, P], FP32, isOutput=False)
    l2w_e = dp("l2w", [P, 2], FP32, isOutput=False)
    l2b_e = dp("l2b", [1, 2], FP32, isOutput=False)
    out_e = dp("out", [P, 2], FP32, isOutput=True)

    # NEFF-baked per-core tables ([NC*128, W]; row = pid*128 + partition)
    idxcat_all = nc.inline_tensor(consts["idxcat"], "idxcat_all")
    degcat_all = nc.inline_tensor(consts["degcat"], "degcat_all")
    slcat_all = nc.inline_tensor(consts["slcat"], "slcat_all")
    iota_all = nc.inline_tensor(consts["iota_f"], "iota_all")

    # +1 row: stripe-local zero row at SPa for self-gather padding
    # layers 1-2 exchange in fp8 (halves AllGather wire + gather traffic);
    # layer 3 feeds the readout directly, keep bf16.
    stripe1 = nc.dram_tensor("stripe1", [SP1 + 1, F], FP8)
    stripe2 = nc.dram_tensor("stripe2", [SP1 + 1, F], FP8)
    stripe3 = nc.dram_tensor("stripe3", [SP2 + 1, F], BF16)
    table1 = nc.dram_tensor("table1", [T1_ROWS, F], FP8, addr_space="Shared")
    table2 = nc.dram_tensor("table2", [T1_ROWS, F], FP8, addr_space="Shared")
    table3 = nc.dram_tensor("table3", [T3_ROWS, F], BF16, addr_space="Shared")
    h1_d = nc.dram_tensor("h1_d", [SP1, F], FP32)
    h2_d = nc.dram_tensor("h2_d", [SP2, F], FP32)

    with tile_mod.TileContext(nc) as tc:
        with (
            tc.tile_pool(name="pp", bufs=1) as pp,
            tc.tile_pool(name="tp", bufs=3) as tp,
        ):
            ident = pp.tile([P, P], FP32, tag="ident")
            make_identity(nc, ident[:])
            identb = pp.tile([P, P], BF16, tag="identb")
            nc.scalar.copy(out=identb[:], in_=ident[:])
            zrow = pp.tile([1, F], BF16, tag="zrow")
            nc.vector.memset(zrow[:], 0.0)
            zrow8 = pp.tile([1, F], FP8, tag="zrow8")
            nc.vector.memset(zrow8[:], 0.0)
            for tbl, rows in ((table1, NC * SP1), (table2, NC * SP1)):
                nc.sync.dma_start(out=tbl[rows : rows + 1, :], in_=zrow8[:])
            nc.sync.dma_start(out=table3[NC * SP2 : NC * SP2 + 1, :], in_=zrow[:])
            for strp, rows in ((stripe1, SP1), (stripe2, SP1)):
                nc.sync.dma_start(out=strp[rows : rows + 1, :], in_=zrow8[:])
            nc.sync.dma_start(out=stripe3[SP2 : SP2 + 1, :], in_=zrow[:])

            def load(ext, shape, tag, dt=FP32):
                t = pp.tile(shape, dt, tag=tag)
                nc.sync.dma_start(out=t[:], in_=ext[:])
                return t

            # per-core slices of the baked tables via pid-offset indirect DMA
            iota_f = load(iota_all, [P, 1], "iota_f")
            assert nc.partition_id_tensor is not None
            pid_u = pp.tile([1, 1], mybir.dt.uint32, tag="pid_u")
            nc.sync.dma_start(out=pid_u[:], in_=nc.partition_id_tensor[0:1, 0:1])
            pid_f = pp.tile([1, 1], FP32, tag="pid_f")
            nc.vector.tensor_copy(out=pid_f[:], in_=pid_u[:])
            ones_r = pp.tile([1, P], FP32, tag="ones_r")
            nc.vector.memset(ones_r[:], 1.0)

            with tc.tile_pool(name="prep_ps", bufs=1, space="PSUM") as prep_ps:
                pid_ps = prep_ps.tile([P, 1], FP32, tag="pid_ps", space="PSUM")
                nc.tensor.matmul(
                    out=pid_ps[:], lhsT=ones_r[:], rhs=pid_f[:], start=True, stop=True
                )
                offf = pp.tile([P, 1], FP32, tag="offf")
                nc.vector.tensor_scalar(
                    out=offf[:], in0=pid_ps[:], scalar1=float(P), scalar2=None,
                    op0=ALU.mult,
                )
                nc.vector.tensor_add(out=offf[:], in0=offf[:], in1=iota_f[:])
                offt = pp.tile([P, 1], I32, tag="offt")
                nc.vector.tensor_copy(out=offt[:], in_=offf[:])

                idxcat = pp.tile([P, NIT], I32, tag="idxcat")
                nc.gpsimd.indirect_dma_start(
                    out=idxcat[:], out_offset=None, in_=idxcat_all[:],
                    in_offset=bass.IndirectOffsetOnAxis(ap=offt[:, 0:1], axis=0),
                )
                degcat = pp.tile([P, CHT], FP32, tag="degcat")
                nc.gpsimd.indirect_dma_start(
                    out=degcat[:], out_offset=None, in_=degcat_all[:],
                    in_offset=bass.IndirectOffsetOnAxis(ap=offt[:, 0:1], axis=0),
                )
                slcat = pp.tile([P, SLT], I32, tag="slcat")
                nc.gpsimd.indirect_dma_start(
                    out=slcat[:], out_offset=None, in_=slcat_all[:],
                    in_offset=bass.IndirectOffsetOnAxis(ap=offt[:, 0:1], axis=0),
                )

                idx1 = idxcat[:, 0:NI1]
                idx2 = idxcat[:, NI1 : NI1 + NI2]
                idx3 = idxcat[:, NI1 + NI2 : NIT]
                sl2i = slcat[:, 0:CH2]
                sl3i = slcat[:, CH2 : CH2 + CH3]
                r1i = slcat[:, CH2 + CH3 : CH2 + CH3 + 1]
                r2i = slcat[:, CH2 + CH3 + 1 : SLT]

                W2 = load(W2_e, [F, F], "W2")
                W3 = load(W3_e, [F, F], "W3")

                def bcast16(t, shape, tag):
                    tb = pp.tile(shape, BF16, tag=tag)
                    nc.scalar.copy(out=tb[:], in_=t[:])
                    return tb

                W1b = bcast16(load(W1_e, [P, F], "W1f"), [P, F], "W1b")
                W2b = bcast16(W2, [F, F], "W2b")
                W3b = bcast16(W3, [F, F], "W3b")

                def brc(ext, n, tag):
                    """[1, n] param -> [P, n] tile via K=1 PE outer product."""
                    row = load(ext, [1, n], tag + "_r")
                    ps_t = prep_ps.tile([P, n], FP32, tag=tag + "_ps", space="PSUM")
                    nc.tensor.matmul(
                        out=ps_t[:], lhsT=ones_r[:], rhs=row[:], start=True, stop=True
                    )
                    t = pp.tile([P, n], FP32, tag=tag)
                    nc.vector.tensor_copy(out=t[:], in_=ps_t[:])
                    return t

                b1 = brc(b1_e, F, "b1")
                b2 = brc(b2_e, F, "b2")
                b3 = brc(b3_e, F, "b3")
                l1b = brc(l1b_e, P, "l1b")
                l2b = brc(l2b_e, 2, "l2b")
                l1w = load(l1w_e, [96, P], "l1w")
                l2w = load(l2w_e, [P, 2], "l2w")

            def dis_of(dslice, ch, tag):
                sq = tp.tile([P, ch], FP32, tag="sq")
                nc.scalar.sqrt(out=sq[:], in_=dslice)
                dis = pp.tile([P, ch], FP32, tag=tag + "_dis")
                nc.vector.reciprocal(out=dis[:], in_=sq[:])
                return dis

            dis1 = dis_of(degcat[:, 0:CH1], CH1, "deg1")
            dis2 = dis_of(degcat[:, CH1 : CH1 + CH2], CH2, "deg2")
            dis3 = dis_of(degcat[:, CH1 + CH2 : CHT], CH3, "deg3")

            def bc_mid(ap2d, nch, width=F):
                # [128, nch] -> [128, nch, width] (inner bcast)
                return ap2d.rearrange("p (c o) -> p c o", o=1).to_broadcast(
                    [P, nch, width]
                )

            def bc_feat(ap2d, nch):
                # [128, F] -> [128, nch, F] (middle bcast)
                return ap2d.rearrange("p (o f) -> p o f", o=1).to_broadcast(
                    [P, nch, F]
                )

            with tc.tile_pool(name="ps", bufs=2, space="PSUM") as ps:

                def transform(kind, in_buf, n_chunks, Wb, dis, hs_tag):
                    """hs = dis * (h @ W); bf16 PE path. in_buf: bf16 xT for
                    kind='x', else f32 node-major h (cast to bf16 here)."""
                    hs = pp.tile([P, n_chunks * F], FP32, tag=hs_tag)
                    if kind == "x":
                        inb = in_buf
                    else:
                        inb = pp.tile([P, n_chunks * F], BF16, tag=hs_tag + "_inb")
                        nc.scalar.copy(out=inb[:], in_=in_buf[:, : n_chunks * F])
                    for g0 in range(0, n_chunks, 4):
                        nch = min(4, n_chunks - g0)
                        if kind == "x":
                            zT_ps = ps.tile([F, 512], FP32, tag="zT", space="PSUM")
                            nc.tensor.matmul(
                                out=zT_ps[:, : nch * P],
                                lhsT=Wb[:],
                                rhs=inb[:, g0 * P : (g0 + nch) * P],
                                start=True, stop=True,
                            )
                        else:
                            hT_ps = ps.tile([F, 512], BF16, tag="hT", space="PSUM")
                            for k in range(nch):
                                nc.tensor.transpose(
                                    out=hT_ps[:, k * P : (k + 1) * P],
                                    in_=inb[:, (g0 + k) * F : (g0 + k + 1) * F],
                                    identity=identb[:],
                                )
                            hT_sb = tp.tile([F, 512], BF16, tag="hT_sb")
                            nc.scalar.copy(
                                out=hT_sb[:, : nch * P], in_=hT_ps[:, : nch * P]
                            )
                            zT_ps = ps.tile([F, 512], FP32, tag="zT", space="PSUM")
                            nc.tensor.matmul(
                                out=zT_ps[:, : nch * P],
                                lhsT=Wb[:],
                                rhs=hT_sb[:, : nch * P],
                                start=True, stop=True,
                            )
                        zT_sb = tp.tile([F, 512], BF16, tag="zT_sb")
                        nc.scalar.copy(out=zT_sb[:, : nch * P], in_=zT_ps[:, : nch * P])
                        zN_ps = ps.tile([P, 4 * F], BF16, tag="zN", space="PSUM")
                        for k in range(nch):
                            nc.tensor.transpose(
                                out=zN_ps[:, k * F : (k + 1) * F],
                                in_=zT_sb[:, k * P : (k + 1) * P],
                                identity=identb[:F, :F],
                            )
                        nc.vector.tensor_tensor(
                            out=hs[:, g0 * F : (g0 + nch) * F].rearrange(
                                "p (c f) -> p c f", c=nch
                            ),
                            in0=zN_ps[:, : nch * F].rearrange("p (c f) -> p c f", c=nch),
                            in1=bc_mid(dis[:, g0 : g0 + nch], nch),
                            op=ALU.mult,
                        )
                    return hs

                def broadcast(gp, hs, hsb_tag, stripe, table, n_chunks, rows, dt8):
                    """write own stripe (casting in SBUF), AllGather."""
                    if dt8:
                        hs_lo = gp.tile([P, n_chunks * F], FP8, tag=hsb_tag)
                        nc.scalar.copy(out=hs_lo[:], in_=hs[:, : n_chunks * F])
                        nc.sync.dma_start(
                            out=stripe[0 : n_chunks * P, :].rearrange(
                                "(p c) f -> p (c f)", p=P
                            ),
                            in_=hs_lo[:],
                        )
                    else:
                        nc.gpsimd.dma_start(
                            out=stripe[0 : n_chunks * P, :].rearrange(
                                "(p c) f -> p (c f)", p=P
                            ),
                            in_=hs[:, : n_chunks * F],
                        )
                    if "collectives" in ABLATE:
                        return None
                    nc.gpsimd.collective_compute(
                        "AllGather",
                        ALU.bypass,
                        replica_groups=[list(range(NC))],
                        ins=[stripe[0 : n_chunks * P, :]],
                        outs=[table[0:rows, :]],
                    )
                    return None

                def self_init(gp, A, stripe, sli, n_chunks, tag, gdt=BF16):
                    """A = own rows gathered from local stripe (pre-AllGather)."""
                    sg = gp.tile([P, n_chunks * F], gdt, tag=tag)
                    for ch in range(n_chunks):
                        nc.gpsimd.indirect_dma_start(
                            out=sg[:, ch * F : (ch + 1) * F],
                            out_offset=None,
                            in_=stripe[:],
                            in_offset=bass.IndirectOffsetOnAxis(
                                ap=sli[:, ch : ch + 1], axis=0
                            ),
                        )
                    nc.scalar.copy(out=A[:], in_=sg[:])

                def aggregate(gp, A, table, idxt, cols_list, ni, gtag, gdt=BF16):
                    if "gathers" in ABLATE:
                        return
                    g = gp.tile([P, ni * F], gdt, tag=gtag)
                    for c in range(ni):
                        gi = nc.gpsimd.indirect_dma_start(
                            out=g[:, c * F : (c + 1) * F],
                            out_offset=None,
                            in_=table[:],
                            in_offset=bass.IndirectOffsetOnAxis(
                                ap=idxt[:, c : c + 1], axis=0
                            ),
                        )
                        if c % 2:
                            gi.ins.queue = "qPoolDynamic1"

                    off = 0
                    for cols in cols_list:
                        nc.vector.tensor_add(
                            out=A[:, : cols * F],
                            in0=A[:, : cols * F],
                            in1=g[:, off * F : (off + cols) * F],
                        )
                        off += cols

                def finish(A, dis, b, n_chunks, h_tag):
                    A3d = A[:].rearrange("p (c f) -> p c f", c=n_chunks)
                    nc.vector.tensor_tensor(
                        out=A3d, in0=A3d, in1=bc_mid(dis[:], n_chunks), op=ALU.mult
                    )
                    nc.vector.tensor_tensor(
                        out=A3d, in0=A3d, in1=bc_feat(b[:], n_chunks), op=ALU.add
                    )
                    h = pp.tile([P, n_chunks * F], FP32, tag=h_tag)
                    nc.scalar.activation(out=h[:], in_=A[:], func=AF.Tanh)
                    return h

                # ---- layer 1 ----
                with tc.tile_pool(name="xp", bufs=1) as xp:
                    xT8 = xp.tile([P, SP1], FP8, tag="xT8")
                    xT = xp.tile([P, SP1], BF16, tag="xT")
                    # chunked load+upcast so transform pipelines with the DMA
                    NL = 4
                    step = ((CH1 + NL - 1) // NL) * P
                    for c0 in range(0, SP1, step):
                        c1 = min(SP1, c0 + step)
                        nc.sync.dma_start(out=xT8[:, c0:c1], in_=xT_e[:, c0:c1])
                        nc.scalar.copy(out=xT[:, c0:c1], in_=xT8[:, c0:c1])
                    hs1 = transform("x", xT, CH1, W1b, dis1, "hs1")
                with tc.tile_pool(name="g1p", bufs=1) as g1p:
                    broadcast(g1p, hs1, "hs1b", stripe1, table1, CH1, NC * SP1, True)
                    aggregate(g1p, hs1, table1, idx1, meta["cols1"], NI1, "G1", FP8)

                    # ---- layer 1 finish + layer 2 transform, staggered per
                    # 4-chunk group: high-index chunks have low in-degree and
                    # complete after the first few gather rounds, so their
                    # finish/transform2/stripe2-write overlaps the remaining
                    # layer-1 gathers, letting AllGather-2 start earlier.
                    h1 = pp.tile([P, CH1 * F], FP32, tag="h1")
                    h1b = pp.tile([P, CH1 * F], BF16, tag="h1b")
                    hs2 = pp.tile([P, CH1 * F], FP32, tag="hs2")
                    hs2_lo = pp.tile([P, CH1 * F], FP8, tag="hs2_lo")
                    for g0 in reversed(range(0, CH1, 4)):
                        nch = min(4, CH1 - g0)
                        sl = slice(g0 * F, (g0 + nch) * F)
                        A3d = hs1[:, sl].rearrange("p (c f) -> p c f", c=nch)
                        nc.vector.tensor_tensor(
                            out=A3d, in0=A3d,
                            in1=bc_mid(dis1[:, g0 : g0 + nch], nch), op=ALU.mult,
                        )
                        nc.vector.tensor_tensor(
                            out=A3d, in0=A3d, in1=bc_feat(b1[:], nch), op=ALU.add,
                        )
                        nc.scalar.activation(
                            out=h1[:, sl], in_=hs1[:, sl], func=AF.Tanh
                        )
                        nc.scalar.copy(out=h1b[:, sl], in_=h1[:, sl])
                        # transform2 on this group: hs2 = dis1 * (h1 @ W2)
                        hT_ps = ps.tile([F, 512], BF16, tag="hT", space="PSUM")
                        for k in range(nch):
                            nc.tensor.transpose(
                                out=hT_ps[:, k * P : (k + 1) * P],
                                in_=h1b[:, (g0 + k) * F : (g0 + k + 1) * F],
                                identity=identb[:],
                            )
                        hT_sb = tp.tile([F, 512], BF16, tag="hT_sb")
                        nc.scalar.copy(
                            out=hT_sb[:, : nch * P], in_=hT_ps[:, : nch * P]
                        )
                        zT_ps = ps.tile([F, 512], FP32, tag="zT", space="PSUM")
                        nc.tensor.matmul(
                            out=zT_ps[:, : nch * P], lhsT=W2b[:],
                            rhs=hT_sb[:, : nch * P], start=True, stop=True,
                        )
                        zT_sb = tp.tile([F, 512], BF16, tag="zT_sb")
                        nc.scalar.copy(out=zT_sb[:, : nch * P], in_=zT_ps[:, : nch * P])
                        zN_ps = ps.tile([P, 4 * F], BF16, tag="zN", space="PSUM")
                        for k in range(nch):
                            nc.tensor.transpose(
                                out=zN_ps[:, k * F : (k + 1) * F],
                                in_=zT_sb[:, k * P : (k + 1) * P],
                                identity=identb[:F, :F],
                            )
                        nc.vector.tensor_tensor(
                            out=hs2[:, sl].rearrange("p (c f) -> p c f", c=nch),
                            in0=zN_ps[:, : nch * F].rearrange("p (c f) -> p c f", c=nch),
                            in1=bc_mid(dis1[:, g0 : g0 + nch], nch),
                            op=ALU.mult,
                        )
                        nc.scalar.copy(out=hs2_lo[:, sl], in_=hs2[:, sl])
                        nc.sync.dma_start(
                            out=stripe2[0 : CH1 * P, :].rearrange(
                                "(p c) f -> p (c f)", p=P
                            )[:, sl],
                            in_=hs2_lo[:, sl],
                        )
                nc.sync.dma_start(
                    out=h1_d[:].rearrange("(p c) f -> p (c f)", p=P), in_=h1[:]
                )
                # readout row-gathers emitted early so they overlap the
                # layer-2/3 gather streams on the Pool queue
                cat = pp.tile([P, 96], FP32, tag="cat")
                nc.gpsimd.indirect_dma_start(
                    out=cat[:, 0:F], out_offset=None, in_=h1_d[:],
                    in_offset=bass.IndirectOffsetOnAxis(ap=r1i[:, 0:1], axis=0),
                )

                # ---- layer 2 ----
                A2 = pp.tile([P, CH2 * F], FP32, tag="A2")
                with tc.tile_pool(name="g2p", bufs=1) as g2p:
                    if "collectives" not in ABLATE:
                        nc.gpsimd.collective_compute(
                            "AllGather",
                            ALU.bypass,
                            replica_groups=[list(range(NC))],
                            ins=[stripe2[0 : CH1 * P, :]],
                            outs=[table2[0 : NC * SP1, :]],
                        )
                    self_init(g2p, A2, stripe2, sl2i, CH2, "sg2", FP8)
                    aggregate(g2p, A2, table2, idx2, meta["cols2"], NI2, "G2", FP8)

                    # layer-2 finish + layer-3 transform, staggered per group
                    # (same trick as layer 1->2; CH2=16 so 4 groups)
                    h2 = pp.tile([P, CH2 * F], FP32, tag="h2")
                    h2b = pp.tile([P, CH2 * F], BF16, tag="h2b")
                    hs3 = pp.tile([P, CH2 * F], FP32, tag="hs3")
                    hs3_lo = pp.tile([P, CH2 * F], BF16, tag="hs3_lo")
                    for g0 in reversed(range(0, CH2, 4)):
                        nch = min(4, CH2 - g0)
                        sl = slice(g0 * F, (g0 + nch) * F)
                        A3d = A2[:, sl].rearrange("p (c f) -> p c f", c=nch)
                        nc.vector.tensor_tensor(
                            out=A3d, in0=A3d,
                            in1=bc_mid(dis2[:, g0 : g0 + nch], nch), op=ALU.mult,
                        )
                        nc.vector.tensor_tensor(
                            out=A3d, in0=A3d, in1=bc_feat(b2[:], nch), op=ALU.add,
                        )
                        nc.scalar.activation(
                            out=h2[:, sl], in_=A2[:, sl], func=AF.Tanh
                        )
                        nc.scalar.copy(out=h2b[:, sl], in_=h2[:, sl])
                        hT_ps = ps.tile([F, 512], BF16, tag="hT", space="PSUM")
                        for k in range(nch):
                            nc.tensor.transpose(
                                out=hT_ps[:, k * P : (k + 1) * P],
                                in_=h2b[:, (g0 + k) * F : (g0 + k + 1) * F],
                                identity=identb[:],
                            )
                        hT_sb = tp.tile([F, 512], BF16, tag="hT_sb")
                        nc.scalar.copy(
                            out=hT_sb[:, : nch * P], in_=hT_ps[:, : nch * P]
                        )
                        zT_ps = ps.tile([F, 512], FP32, tag="zT", space="PSUM")
                        nc.tensor.matmul(
                            out=zT_ps[:, : nch * P], lhsT=W3b[:],
                            rhs=hT_sb[:, : nch * P], start=True, stop=True,
                        )
                        zT_sb = tp.tile([F, 512], BF16, tag="zT_sb")
                        nc.scalar.copy(out=zT_sb[:, : nch * P], in_=zT_ps[:, : nch * P])
                        zN_ps = ps.tile([P, 4 * F], BF16, tag="zN", space="PSUM")
                        for k in range(nch):
                            nc.tensor.transpose(
                                out=zN_ps[:, k * F : (k + 1) * F],
                                in_=zT_sb[:, k * P : (k + 1) * P],
                                identity=identb[:F, :F],
                            )
                        nc.vector.tensor_tensor(
                            out=hs3[:, sl].rearrange("p (c f) -> p c f", c=nch),
                            in0=zN_ps[:, : nch * F].rearrange("p (c f) -> p c f", c=nch),
                            in1=bc_mid(dis2[:, g0 : g0 + nch], nch),
                            op=ALU.mult,
                        )
                        nc.scalar.copy(out=hs3_lo[:, sl], in_=hs3[:, sl])
                        nc.sync.dma_start(
                            out=stripe3[0 : CH2 * P, :].rearrange(
                                "(p c) f -> p (c f)", p=P
                            )[:, sl],
                            in_=hs3_lo[:, sl],
                        )
                nc.sync.dma_start(
                    out=h2_d[:].rearrange("(p c) f -> p (c f)", p=P), in_=h2[:]
                )
                nc.gpsimd.indirect_dma_start(
                    out=cat[:, F : 2 * F], out_offset=None, in_=h2_d[:],
                    in_offset=bass.IndirectOffsetOnAxis(ap=r2i[:, 0:1], axis=0),
                )

                # ---- layer 3 ----
                A3 = pp.tile([P, CH3 * F], FP32, tag="A3")
                with tc.tile_pool(name="g3p", bufs=1) as g3p:
                    if "collectives" not in ABLATE:
                        nc.gpsimd.collective_compute(
                            "AllGather",
                            ALU.bypass,
                            replica_groups=[list(range(NC))],
                            ins=[stripe3[0 : CH2 * P, :]],
                            outs=[table3[0 : NC * SP2, :]],
                        )
                    self_init(g3p, A3, stripe3, sl3i, CH3, "sg3")
                    aggregate(g3p, A3, table3, idx3, meta["cols3"], NI3, "G3")
                    h3 = finish(A3, dis3, b3, CH3, "h3")

            # ---- readout ----
            with (
                tc.tile_pool(name="rp", bufs=1, space="PSUM") as rp,
                tc.tile_pool(name="rsb", bufs=1) as rsb,
            ):
                nc.vector.tensor_copy(out=cat[:, 2 * F : 3 * F], in_=h3[:, :F])

                cT_ps = rp.tile([96, P], FP32, tag="cT", space="PSUM")
                nc.tensor.transpose(out=cT_ps[:], in_=cat[:], identity=ident[:])
                cT = rsb.tile([96, P], FP32, tag="cTs")
                nc.scalar.copy(out=cT[:], in_=cT_ps[:])
                hid_ps = rp.tile([P, P], FP32, tag="hid", space="PSUM")
                nc.tensor.matmul(out=hid_ps[:], lhsT=cT[:], rhs=l1w[:], start=True, stop=True)
                hid = rsb.tile([P, P], FP32, tag="hids")
                nc.vector.tensor_add(out=hid[:], in0=hid_ps[:], in1=l1b[:])
                hidr = rsb.tile([P, P], FP32, tag="hidr")
                nc.scalar.activation(out=hidr[:], in_=hid[:], func=AF.Relu)
                hT_ps = rp.tile([P, P], FP32, tag="hT2", space="PSUM")
                nc.tensor.transpose(out=hT_ps[:], in_=hidr[:], identity=ident[:])
                hT = rsb.tile([P, P], FP32, tag="hT2s")
                nc.scalar.copy(out=hT[:], in_=hT_ps[:])
                lg_ps = rp.tile([P, 2], FP32, tag="lg", space="PSUM")
                nc.tensor.matmul(out=lg_ps[:], lhsT=hT[:], rhs=l2w[:], start=True, stop=True)
                lg = rsb.tile([P, 2], FP32, tag="lgs")
                nc.vector.tensor_add(out=lg[:], in0=lg_ps[:], in1=l2b[:])
                m = rsb.tile([P, 1], FP32, tag="m")
                nc.vector.tensor_reduce(out=m[:], in_=lg[:], axis=mybir.AxisListType.X, op=ALU.max)
                t = rsb.tile([P, 2], FP32, tag="t")
                nc.vector.tensor_scalar(out=t[:], in0=lg[:], scalar1=m[:], scalar2=None, op0=ALU.subtract)
                e = rsb.tile([P, 2], FP32, tag="e")
                nc.scalar.activation(out=e[:], in_=t[:], func=AF.Exp)
                s = rsb.tile([P, 1], FP32, tag="s")
                nc.vector.tensor_reduce(out=s[:], in_=e[:], axis=mybir.AxisListType.X, op=ALU.add)
                ls = rsb.tile([P, 1], FP32, tag="ls")
                nc.scalar.activation(out=ls[:], in_=s[:], func=AF.Ln)
                o = rsb.tile([P, 2], FP32, tag="o")
                nc.vector.tensor_scalar(out=o[:], in0=t[:], scalar1=ls[:], scalar2=None, op0=ALU.subtract)
                nc.sync.dma_start(out=out_e[:], in_=o[:])

    _split_waits(nc)
    return nc


# ---------------------------------------------------------------------------
# entry point
# ---------------------------------------------------------------------------

_CACHE = {}


def _get_runner(meta, consts):
    import hashlib

    h = hashlib.sha1()
    for name in ("idxcat", "degcat", "slcat", "iota_f"):
        h.update(consts[name].tobytes())
    key = (
        meta["CH1"], meta["CH2"], meta["CH3"], meta["NI1"], meta["NI2"], meta["NI3"],
        tuple(meta["cols1"]), tuple(meta["cols2"]), tuple(meta["cols3"]),
        h.hexdigest(),
    )
    if key not in _CACHE:
        nc = _build(meta, consts)
        _CACHE[key] = _SpmdRunner(nc)
    return _CACHE[key]


def kernel(x, edge_index, batch, num_graphs,
           W1, b1, W2, b2, W3, b3, lin1_w, lin1_b, lin2_w, lin2_b):
    x = np.asarray(x, np.float32)
    edge_index = np.asarray(edge_index)
    batch = np.asarray(batch)
    G = int(np.asarray(num_graphs))
    W1 = np.asarray(W1, np.float32)
    W2 = np.asarray(W2, np.float32)
    W3 = np.asarray(W3, np.float32)
    b1 = np.asarray(b1, np.float32)
    b2 = np.asarray(b2, np.float32)
    b3 = np.asarray(b3, np.float32)
    lin1_w = np.asarray(lin1_w, np.float32)
    lin1_b = np.asarray(lin1_b, np.float32)
    lin2_w = np.asarray(lin2_w, np.float32)
    lin2_b = np.asarray(lin2_b, np.float32)

    meta, per_core, slot_graphs = _preprocess(x, edge_index, batch, G)

    # stack per-core tables -> NEFF-baked constants [NC*128, W]
    def stack(key_list, dtype):
        per = [
            np.concatenate([per_core[c][k] for k in key_list], axis=1)
            for c in range(NC)
        ]
        return np.ascontiguousarray(np.stack(per, axis=0).reshape(NC * P, -1), dtype)

    consts = dict(
        idxcat=stack(["idx1", "idx2", "idx3"], np.int32),
        degcat=stack(["deg1", "deg2", "deg3"], np.float32),
        slcat=stack(["sl2", "sl3", "r1", "r2"], np.int32),
        iota_f=np.arange(P, dtype=np.float32).reshape(P, 1),
    )
    runner = _get_runner(meta, consts)

    fp8 = ml_dtypes.float8_e4m3
    in_maps = []
    for c in range(NC):
        pc = per_core[c]
        in_maps.append(
            dict(
                xT=pc["xT"].astype(fp8),
                W1=W1, W2=W2, W3=W3,
                b1=b1[None, :], b2=b2[None, :], b3=b3[None, :],
                l1w=lin1_w.astype(np.float32),
                l1b=lin1_b[None, :],
                l2w=lin2_w.astype(np.float32),
                l2b=lin2_b[None, :],
            )
        )

    args = runner.stage(in_maps)
    outs = runner.run_staged(args)
    res = runner.results(outs)

    logits = np.zeros((G, 2), np.float32)
    for c in range(NC):
        gids = slot_graphs[c]
        logits[gids] = res[c]["out"][: len(gids)]

    # expose for test.py timing
    kernel._last = (runner, args)
    return logits



# revision 27
# speedup vs baseline: 1.2316x; 1.2316x over previous
"""3-layer GCN (BaseNet) on 8 Trainium2 NeuronCores.

Strategy (self-contained; hardcoded for N=100000, E=1.6M, D=128, LAT=32x3, G=1000):
 - Nodes sharded contiguously across 8 cores (12500/core); each core owns all
   edges whose dst lies in its shard.
 - Per core, shard nodes are sorted by in-degree so the edge aggregation
   becomes "rounds": round r gathers hs[src_r(n)] for the prefix of nodes with
   indeg > r, landing slot-aligned with the accumulator -> plain DVE adds,
   no scatter needed.
 - Gathers are 128-row indirect DMAs from a per-core HBM replica of the
   scaled feature table hs = deg^-1/2 * (h @ W); replicas built by AllGather.
 - Math identity used: agg + self = dis * (sum_e hs[src] + hs[own]); the
   edge coefficient dis[src]*dis[dst] factors into per-node pre/post scales.
 - Backward slicing: layer-2 output is only needed at readout nodes and their
   in-neighbors (~17K nodes); layer-3 only at the 1000 readout nodes. Cuts
   gather volume ~2.7x.
 - Readout (first node of each graph) + 2-layer MLP + log_softmax on device.

Perf notes (why it looks the way it does):
 - Per-dispatch PJRT arg staging costs ~90-130 us/MB/core, so all
   topology-derived tables (gather indices, degree grids, self/readout maps)
   are NEFF-baked inline constants; each core fetches its slice of the
   [8*128, W] stacked tables with one partition_id-offset indirect DMA.
   Only x (as fp8_e4m3, halving the dominant upload) and the model weights
   are true per-dispatch inputs.
 - Layer-1/2 feature tables (stripes + AllGather replicas) are fp8_e4m3:
   halves collective wire and gather HBM traffic; DVE adds consume fp8
   operands directly. Layer-3 stays bf16 (feeds the readout, error budget).
 - Layer-1 finish + layer-2 transform + stripe-2 writes are emitted per
   4-chunk group in reverse chunk order: low-indegree chunks complete after
   the first few gather rounds, so their tail work overlaps the remaining
   layer-1 gathers and AllGather-2 starts earlier.
 - Each indirect gather (128 rows) costs ~0.3-1 us fixed on the Pool engine
   (SWDGE); instruction count is the floor. Batched alternatives were probed
   on HW and are unusable here: multi-index offset APs mis-unroll in walrus,
   and dma_gather/ap_gather (custom GPSIMD ucode) fail neuronxcc codegen.
"""
import os
import sys

for _p in ("/opt/trn_rl_repo", "/root/.axon_site/_ro/trn_rl_repo"):
    if os.path.isdir(_p) and _p not in sys.path:
        sys.path.insert(0, _p)

import ml_dtypes
import numpy as np

import concourse.bass as bass
import concourse.mybir as mybir
import concourse.tile as tile_mod
from concourse.masks import make_identity
from concourse.vector_clock import ScopedClock

NC = 8
F = 32          # latent feature dim
P = 128

FP32 = mybir.dt.float32
I32 = mybir.dt.int32
AF = mybir.ActivationFunctionType
ALU = mybir.AluOpType

# ---------------------------------------------------------------------------
# walrus compat: this build rejects >1 sem wait per instruction. Spread the
# Tile drain's waits and any multi-wait instruction across nop carriers.
# ---------------------------------------------------------------------------

def _patched_drain_and_barrier(self, tick_clock, wait_clock):
    probe = self.nc.sync.nop(nofuse=True, hint="drain_wait_carrier")
    wait_clock.add_sem_waits(probe.ins, ScopedClock({None: tick_clock.global_clock}))
    si = probe.ins.sync_info
    waits = list(si.on_wait) if si is not None else []
    probe.ins.sync_info = mybir.SyncInfo(
        on_wait=waits[:1], on_update=list(si.on_update) if si is not None else []
    )
    for k in range(1, len(waits)):
        extra = self.nc.sync.nop(nofuse=True, hint=f"drain_wait_carrier_{k}")
        extra.ins.sync_info = mybir.SyncInfo(on_wait=waits[k : k + 1], on_update=[])
    self.nc.sync.drain()
    self.nc.all_engine_barrier()
    assert self.sems is not None
    popped = self.nc._tile_sem_poison_stack.pop()
    assert popped is self._sem_poison
    self.nc.clear_and_free_semaphores(list(self.sems.allocated().values()))
    self.nc.all_engine_barrier()


tile_mod.TileContext._drain_and_barrier = _patched_drain_and_barrier


def _split_waits(nc, max_waits=1):
    ctr = 0
    for fn in nc.m.functions:
        for bb in fn.blocks:
            out = []
            changed = False
            for inst in bb.instructions:
                si = inst.sync_info
                n = len(si.on_wait) if si is not None else 0
                if n > max_waits:
                    waits = list(si.on_wait)
                    keep = waits[-max_waits:]
                    extra = waits[:-max_waits]
                    for k in range(0, len(extra), max_waits):
                        nop = mybir.InstNoOp(
                            name=f"waitnop-{ctr}",
                            engine=inst.engine,
                            bass_nofuse=True,
                            sync_info=mybir.SyncInfo(
                                on_wait=extra[k : k + max_waits], on_update=[]
                            ),
                        )
                        ctr += 1
                        out.append(nop)
                    inst.sync_info = mybir.SyncInfo(
                        on_wait=keep, on_update=list(si.on_update)
                    )
                    changed = True
                out.append(inst)
            if changed:
                bb.instructions = out
    return ctr


# ---------------------------------------------------------------------------
# SPMD runner (PJRT/axon), build once / run many.
# ---------------------------------------------------------------------------

class _SpmdRunner:
    def __init__(self, nc, n_cores=NC):
        import jax
        from jax.sharding import Mesh, NamedSharding, PartitionSpec
        from jax.experimental.shard_map import shard_map
        from concourse.bass2jax import (
            _bass_exec_p,
            install_neuronx_cc_hook,
            partition_id_tensor,
        )

        self.jax = jax
        install_neuronx_cc_hook()
        self.n_cores = n_cores
        partition_name = nc.partition_id_tensor.name if nc.partition_id_tensor else None

        in_names, out_names, out_avals = [], [], []
        for alloc in nc.m.functions[0].allocations:
            if not isinstance(alloc, mybir.MemoryLocationSet):
                continue
            name = alloc.memorylocations[0].name
            if alloc.kind == "ExternalInput":
                if name != partition_name:
                    in_names.append(name)
            elif alloc.kind == "ExternalOutput":
                out_avals.append(
                    jax.core.ShapedArray(
                        tuple(alloc.tensor_shape), mybir.dt.np(alloc.dtype)
                    )
                )
                out_names.append(name)
        self.in_names, self.out_names, self.out_avals = in_names, out_names, out_avals
        n_params = len(in_names)
        all_in = in_names + out_names + ([partition_name] if partition_name else [])

        def _body(*args):
            operands = list(args)
            if partition_name is not None:
                operands.append(partition_id_tensor())
            return tuple(
                _bass_exec_p.bind(
                    *operands,
                    out_avals=tuple(out_avals),
                    in_names=tuple(all_in),
                    out_names=tuple(out_names),
                    lowering_input_output_aliases=(),
                    sim_require_finite=True,
                    sim_require_nnan=True,
                    nc=nc,
                )
            )

        devices = jax.devices()[:n_cores]
        assert len(devices) == n_cores, f"need {n_cores} cores, saw {len(jax.devices())}"
        mesh = Mesh(np.asarray(devices), ("core",))
        self.sharded = jax.jit(
            shard_map(
                _body,
                mesh=mesh,
                in_specs=(PartitionSpec("core"),) * (n_params + len(out_names)),
                out_specs=(PartitionSpec("core"),) * len(out_names),
                check_rep=False,
            ),
            keep_unused=True,
        )
        self.sharding = NamedSharding(mesh, PartitionSpec("core"))

    def stage(self, in_maps):
        args = []
        for name in self.in_names:
            cat = np.concatenate(
                [np.ascontiguousarray(in_maps[c][name]) for c in range(self.n_cores)],
                axis=0,
            )
            args.append(self.jax.device_put(cat, self.sharding))
        for av in self.out_avals:
            z = np.zeros((self.n_cores * av.shape[0], *av.shape[1:]), av.dtype)
            args.append(self.jax.device_put(z, self.sharding))
        return args

    def run_staged(self, args):
        outs = self.sharded(*args)
        self.jax.block_until_ready(outs)
        return outs

    def results(self, outs):
        n = self.n_cores
        return [
            {
                name: np.asarray(outs[i]).reshape(n, *self.out_avals[i].shape)[c]
                for i, name in enumerate(self.out_names)
            }
            for c in range(n)
        ]


# ---------------------------------------------------------------------------
# Host preprocessing
# ---------------------------------------------------------------------------

def _cdiv(a, b):
    return -(-a // b)


class _AggStruct:
    """Slot-aligned gather rounds for one layer, unified across cores."""

    def __init__(self, active, indeg, shard):
        N = active.shape[0]
        self.N = N
        self.shard = shard
        self.pos = np.full(N, -1, np.int64)
        self.perm = []
        counts = []
        for c in range(NC):
            nodes = np.flatnonzero(active[c * shard : (c + 1) * shard]) + c * shard
            order = np.argsort(-indeg[nodes], kind="stable")
            pc = nodes[order]
            self.perm.append(pc)
            self.pos[pc] = np.arange(len(pc))
            counts.append(len(pc))
        self.counts = counts
        self.chunks = max(1, _cdiv(max(counts), P))
        SPa = self.chunks * P
        self.SPa = SPa

        # rounds: cols per round, unified (max over cores)
        R = 0
        percore_d = []
        for c in range(NC):
            d = indeg[self.perm[c]]  # sorted desc
            R = max(R, int(d[0]) if len(d) else 0)
            percore_d.append(d)
        cols = []
        for r in range(R):
            m = 0
            for c in range(NC):
                nr = int(np.count_nonzero(percore_d[c] > r))
                m = max(m, _cdiv(nr, P))
            if m == 0:
                break
            cols.append(m)
        self.cols = cols
        self.NI = sum(cols)

    def build_idx(self, e_src, e_dst, t_row, Z):
        off = np.zeros(len(self.cols) + 1, np.int64)
        off[1:] = np.cumsum(self.cols)
        idx = np.full((NC, self.NI, P), Z, np.int32)
        if len(e_dst) and self.NI:
            key = (e_dst // self.shard) * self.SPa + self.pos[e_dst]
            order_e = np.argsort(key, kind="stable")
            sk = key[order_e]
            gs = np.r_[0, np.flatnonzero(np.diff(sk)) + 1]
            seq = np.arange(len(sk)) - np.repeat(gs, np.diff(np.r_[gs, len(sk)]))
            c_e = sk // self.SPa
            k_e = sk % self.SPa
            instr = off[seq] + (k_e // P)
            idx[c_e, instr, k_e % P] = t_row[e_src[order_e]].astype(np.int32)
        return idx  # [NC, NI, 128]

    def self_idx(self, t_row, Z):
        """[NC, chunks, 128] gather-own-row indices (for accumulator init)."""
        out = np.full((NC, self.chunks, P), Z, np.int32)
        for c in range(NC):
            k = np.arange(len(self.perm[c]))
            out[c, k // P, k % P] = t_row[self.perm[c]].astype(np.int32)
        return out

    def grid(self, vals, pad, dtype=np.float32):
        """[NC, 128, chunks] per-slot values in (p, ch) layout."""
        g = np.full((NC, P, self.chunks), pad, dtype)
        for c in range(NC):
            k = np.arange(len(self.perm[c]))
            g[c, k % P, k // P] = vals[self.perm[c]]
        return g

    def local_row(self, nodes):
        """flat stripe row (p*chunks + ch) of nodes within their own core."""
        pos = self.pos[nodes]
        return (pos % P) * self.chunks + pos // P

    def table_row(self, nodes):
        """global table row across core stripes."""
        c = np.asarray(nodes) // self.shard
        pos = self.pos[nodes]
        return c * self.SPa + (pos % P) * self.chunks + pos // P


def _preprocess(x, edge_index, batch, num_graphs):
    N = x.shape[0]
    E = edge_index.shape[1]
    G = int(num_graphs)
    shard = N // NC
    assert N % NC == 0
    src = edge_index[0].astype(np.int64)
    dst = edge_index[1].astype(np.int64)
    indeg = np.bincount(dst, minlength=N)
    deg = (indeg + 1).astype(np.float32)

    idxg = np.searchsorted(batch, np.arange(G, dtype=np.int64))

    # backward slicing
    act3 = np.zeros(N, bool)
    act3[idxg] = True
    m3 = act3[dst]
    act2 = np.zeros(N, bool)
    act2[src[m3]] = True
    act2[idxg] = True
    m2 = act2[dst]
    act1 = np.ones(N, bool)

    indeg2 = np.bincount(dst[m2], minlength=N)
    indeg3 = np.bincount(dst[m3], minlength=N)

    # layer 1: table covers all nodes, but only act1 nodes (h1 consumers:
    # sources of layer-2 edges + readout) aggregate. Sorting each shard as
    # [act1 by indeg desc, then inactive] keeps the accumulator == hs1 buffer
    # (self term free) while rounds cover only the active prefix.
    act1 = np.zeros(N, bool)
    act1[src[m2]] = True
    act1[idxg] = True
    m1 = act1[dst]
    indeg1m = np.where(act1, indeg, 0)
    a1 = _AggStruct(np.ones(N, bool), indeg1m, shard)
    Z1 = NC * a1.SPa
    t1 = a1.table_row(np.arange(N))
    idx1_raw = a1.build_idx(src[m1], dst[m1], t1, Z1)

    # layer 2: accumulate only at act2 nodes; table2 is full (t1 coords)
    a2 = _AggStruct(act2, indeg2, shard)
    idx2_raw = a2.build_idx(src[m2], dst[m2], t1, Z1)
    # layer 3: accumulate at readout nodes; table3 covers act2 (perm2 stripes)
    t3 = np.zeros(N, np.int64)
    a2n = np.flatnonzero(act2)
    t3[a2n] = a2.table_row(a2n)
    Z3 = NC * a2.SPa
    a3 = _AggStruct(act3, indeg3, shard)
    idx3_raw = a3.build_idx(src[m3], dst[m3], t3, Z3)

    # stripe-local self rows (gathered from the core's own stripe, pre-AllGather)
    # local stripe row of own node n = t(n) - c*SPa ; zero row = SPa (extra row)
    selfl2 = np.full((NC, a2.chunks, P), a1.SPa, np.int32)
    for c in range(NC):
        k = np.arange(len(a2.perm[c]))
        selfl2[c, k // P, k % P] = (t1[a2.perm[c]] - c * a1.SPa).astype(np.int32)
    selfl3 = np.full((NC, a3.chunks, P), a2.SPa, np.int32)
    for c in range(NC):
        k = np.arange(len(a3.perm[c]))
        selfl3[c, k // P, k % P] = (t3[a3.perm[c]] - c * a2.SPa).astype(np.int32)

    # grids
    deg1_g = a1.grid(deg, 1.0)
    deg2_g = a2.grid(deg, 1.0)
    deg3_g = a3.grid(deg, 1.0)

    # x transposed+permuted per core: [128, SPa1]
    xT = np.zeros((NC, P, a1.SPa), np.float32)
    for c in range(NC):
        xT[c, :, : len(a1.perm[c])] = x[a1.perm[c]].T

    # readout: slot p of core c = a3.perm[c][p]
    r1_idx = np.zeros((NC, P, 1), np.int32)
    r2_idx = np.zeros((NC, P, 1), np.int32)
    graph_of = -np.ones(N, np.int64)
    graph_of[idxg] = np.arange(G)
    slot_graphs = []
    for c in range(NC):
        nodes = a3.perm[c]
        assert len(nodes) <= P, f"core {c} has {len(nodes)} graphs (>128)"
        r1_idx[c, : len(nodes), 0] = a1.local_row(nodes)
        r2_idx[c, : len(nodes), 0] = a2.local_row(nodes)
        slot_graphs.append(graph_of[nodes])

    # gather idx arrays, transposed to [128, NI] for contiguous DMA
    def tr(a):  # [NC, NI, 128] -> [NC, 128, NI]
        return np.ascontiguousarray(np.transpose(a, (0, 2, 1)))

    idx1 = tr(idx1_raw)
    idx2 = tr(idx2_raw)
    idx3 = tr(idx3_raw)
    sl2 = tr(selfl2)
    sl3 = tr(selfl3)
    if idx1.shape[2] == 0:  # degenerate: no edges at all
        idx1 = np.full((NC, P, 1), Z1, np.int32)
        a1.cols = [1]
    if idx2.shape[2] == 0:
        idx2 = np.full((NC, P, 1), Z1, np.int32)
        a2.cols = [1]
    if idx3.shape[2] == 0:
        idx3 = np.full((NC, P, 1), Z3, np.int32)
        a3.cols = [1]

    meta = dict(
        CH1=a1.chunks, CH2=a2.chunks, CH3=a3.chunks,
        SP1=a1.SPa, SP2=a2.SPa, SP3=a3.SPa,
        cols1=a1.cols,
        cols2=a2.cols,
        cols3=a3.cols,
        NI1=idx1.shape[2], NI2=idx2.shape[2], NI3=idx3.shape[2],
        G=G,
    )
    per_core = [
        dict(
            xT=xT[c], deg1=deg1_g[c], deg2=deg2_g[c], deg3=deg3_g[c],
            idx1=idx1[c], idx2=idx2[c], idx3=idx3[c],
            sl2=sl2[c], sl3=sl3[c],
            r1=r1_idx[c], r2=r2_idx[c],
        )
        for c in range(NC)
    ]
    return meta, per_core, slot_graphs


# ---------------------------------------------------------------------------
# Device program
# ---------------------------------------------------------------------------

ABLATE = set()  # {"collectives", "gathers", "transforms"} for timing experiments


def _build(meta, consts):
    CH1, CH2, CH3 = meta["CH1"], meta["CH2"], meta["CH3"]
    SP1, SP2 = meta["SP1"], meta["SP2"]
    NI1, NI2, NI3 = meta["NI1"], meta["NI2"], meta["NI3"]
    T1_ROWS = NC * SP1 + P
    T3_ROWS = NC * SP2 + P
    BF16 = mybir.dt.bfloat16
    FP8 = mybir.dt.float8e4
    NIT = NI1 + NI2 + NI3
    CHT = CH1 + CH2 + CH3
    SLT = CH2 + CH3 + 2

    # 4x SWDGE descriptor ring (4096 descs = 32 indirect gathers in
    # flight vs default 8): the gather stream is completion-latency
    # bound through this ring; 64KB measured best (96KB regressed)
    nc = bass.Bass(dynamic_dma_scratch_size=65536)
    dp = nc.declare_dram_parameter
    xT_e = dp("xT", [P, SP1], FP8, isOutput=False)
    W1_e = dp("W1", [P, F], FP32, isOutput=False)
    W2_e = dp("W2", [F, F], FP32, isOutput=False)
    W3_e = dp("W3", [F, F], FP32, isOutput=False)
    b1_e = dp("b1", [1, F], FP32, isOutput=False)
    b2_e = dp("b2", [1, F], FP32, isOutput=False)
    b3_e = dp("b3", [1, F], FP32, isOutput=False)
    l1w_e = dp("l1w", [96, P], FP32, isOutput=False)
    l1b_e = dp("l1b", [1, P], FP32, isOutput=False)
    l2w_e = dp("l2w", [P, 2], FP32, isOutput=False)
    l2b_e = dp("l2b", [1, 2], FP32, isOutput=False)
    out_e = dp("out", [P, 2], FP32, isOutput=True)

    # NEFF-baked per-core tables ([NC*128, W]; row = pid*128 + partition)
    idxcat_all = nc.inline_tensor(consts["idxcat"], "idxcat_all")
    degcat_all = nc.inline_tensor(consts["degcat"], "degcat_all")
    slcat_all = nc.inline_tensor(consts["slcat"], "slcat_all")
    iota_all = nc.inline_tensor(consts["iota_f"], "iota_all")

    # +1 row: stripe-local zero row at SPa for self-gather padding
    # layers 1-2 exchange in fp8 (halves AllGather wire + gather traffic);
    # layer 3 feeds the readout directly, keep bf16.
    stripe1 = nc.dram_tensor("stripe1", [SP1 + 1, F], FP8)
    stripe2 = nc.dram_tensor("stripe2", [SP1 + 1, F], FP8)
    stripe3 = nc.dram_tensor("stripe3", [SP2 + 1, F], BF16)
    table1 = nc.dram_tensor("table1", [T1_ROWS, F], FP8, addr_space="Shared")
    table2 = nc.dram_tensor("table2", [T1_ROWS, F], FP8, addr_space="Shared")
    table3 = nc.dram_tensor("table3", [T3_ROWS, F], BF16, addr_space="Shared")
    h1_d = nc.dram_tensor("h1_d", [SP1, F], FP32)
    h2_d = nc.dram_tensor("h2_d", [SP2, F], FP32)

    with tile_mod.TileContext(nc) as tc:
        with (
            tc.tile_pool(name="pp", bufs=1) as pp,
            tc.tile_pool(name="tp", bufs=3) as tp,
        ):
            ident = pp.tile([P, P], FP32, tag="ident")
            make_identity(nc, ident[:])
            identb = pp.tile([P, P], BF16, tag="identb")
            nc.scalar.copy(out=identb[:], in_=ident[:])
            zrow = pp.tile([1, F], BF16, tag="zrow")
            nc.vector.memset(zrow[:], 0.0)
            zrow8 = pp.tile([1, F], FP8, tag="zrow8")
            nc.vector.memset(zrow8[:], 0.0)
            for tbl, rows in ((table1, NC * SP1), (table2, NC * SP1)):
                nc.sync.dma_start(out=tbl[rows : rows + 1, :], in_=zrow8[:])
            nc.sync.dma_start(out=table3[NC * SP2 : NC * SP2 + 1, :], in_=zrow[:])
            for strp, rows in ((stripe1, SP1), (stripe2, SP1)):
                nc.sync.dma_start(out=strp[rows : rows + 1, :], in_=zrow8[:])
            nc.sync.dma_start(out=stripe3[SP2 : SP2 + 1, :], in_=zrow[:])

            def load(ext, shape, tag, dt=FP32):
                t = pp.tile(shape, dt, tag=tag)
                nc.sync.dma_start(out=t[:], in_=ext[:])
                return t

            # per-core slices of the baked tables via pid-offset indirect DMA
            iota_f = load(iota_all, [P, 1], "iota_f")
            assert nc.partition_id_tensor is not None
            pid_u = pp.tile([1, 1], mybir.dt.uint32, tag="pid_u")
            nc.sync.dma_start(out=pid_u[:], in_=nc.partition_id_tensor[0:1, 0:1])
            pid_f = pp.tile([1, 1], FP32, tag="pid_f")
            nc.vector.tensor_copy(out=pid_f[:], in_=pid_u[:])
            ones_r = pp.tile([1, P], FP32, tag="ones_r")
            nc.vector.memset(ones_r[:], 1.0)

            with tc.tile_pool(name="prep_ps", bufs=1, space="PSUM") as prep_ps:
                pid_ps = prep_ps.tile([P, 1], FP32, tag="pid_ps", space="PSUM")
                nc.tensor.matmul(
                    out=pid_ps[:], lhsT=ones_r[:], rhs=pid_f[:], start=True, stop=True
                )
                offf = pp.tile([P, 1], FP32, tag="offf")
                nc.vector.tensor_scalar(
                    out=offf[:], in0=pid_ps[:], scalar1=float(P), scalar2=None,
                    op0=ALU.mult,
                )
                nc.vector.tensor_add(out=offf[:], in0=offf[:], in1=iota_f[:])
                offt = pp.tile([P, 1], I32, tag="offt")
                nc.vector.tensor_copy(out=offt[:], in_=offf[:])

                idxcat = pp.tile([P, NIT], I32, tag="idxcat")
                nc.gpsimd.indirect_dma_start(
                    out=idxcat[:], out_offset=None, in_=idxcat_all[:],
                    in_offset=bass.IndirectOffsetOnAxis(ap=offt[:, 0:1], axis=0),
                )
                degcat = pp.tile([P, CHT], FP32, tag="degcat")
                nc.gpsimd.indirect_dma_start(
                    out=degcat[:], out_offset=None, in_=degcat_all[:],
                    in_offset=bass.IndirectOffsetOnAxis(ap=offt[:, 0:1], axis=0),
                )
                slcat = pp.tile([P, SLT], I32, tag="slcat")
                nc.gpsimd.indirect_dma_start(
                    out=slcat[:], out_offset=None, in_=slcat_all[:],
                    in_offset=bass.IndirectOffsetOnAxis(ap=offt[:, 0:1], axis=0),
                )

                idx1 = idxcat[:, 0:NI1]
                idx2 = idxcat[:, NI1 : NI1 + NI2]
                idx3 = idxcat[:, NI1 + NI2 : NIT]
                sl2i = slcat[:, 0:CH2]
                sl3i = slcat[:, CH2 : CH2 + CH3]
                r1i = slcat[:, CH2 + CH3 : CH2 + CH3 + 1]
                r2i = slcat[:, CH2 + CH3 + 1 : SLT]

                W2 = load(W2_e, [F, F], "W2")
                W3 = load(W3_e, [F, F], "W3")

                def bcast16(t, shape, tag):
                    tb = pp.tile(shape, BF16, tag=tag)
                    nc.scalar.copy(out=tb[:], in_=t[:])
                    return tb

                W1b = bcast16(load(W1_e, [P, F], "W1f"), [P, F], "W1b")
                W2b = bcast16(W2, [F, F], "W2b")
                W3b = bcast16(W3, [F, F], "W3b")

                def brc(ext, n, tag):
                    """[1, n] param -> [P, n] tile via K=1 PE outer product."""
                    row = load(ext, [1, n], tag + "_r")
                    ps_t = prep_ps.tile([P, n], FP32, tag=tag + "_ps", space="PSUM")
                    nc.tensor.matmul(
                        out=ps_t[:], lhsT=ones_r[:], rhs=row[:], start=True, stop=True
                    )
                    t = pp.tile([P, n], FP32, tag=tag)
                    nc.vector.tensor_copy(out=t[:], in_=ps_t[:])
                    return t

                b1 = brc(b1_e, F, "b1")
                b2 = brc(b2_e, F, "b2")
                b3 = brc(b3_e, F, "b3")
                l1b = brc(l1b_e, P, "l1b")
                l2b = brc(l2b_e, 2, "l2b")
                l1w = load(l1w_e, [96, P], "l1w")
                l2w = load(l2w_e, [P, 2], "l2w")

            def dis_of(dslice, ch, tag):
                sq = tp.tile([P, ch], FP32, tag="sq")
                nc.scalar.sqrt(out=sq[:], in_=dslice)
                dis = pp.tile([P, ch], FP32, tag=tag + "_dis")
                nc.vector.reciprocal(out=dis[:], in_=sq[:])
                return dis

            dis1 = dis_of(degcat[:, 0:CH1], CH1, "deg1")
            dis2 = dis_of(degcat[:, CH1 : CH1 + CH2], CH2, "deg2")
            dis3 = dis_of(degcat[:, CH1 + CH2 : CHT], CH3, "deg3")

            def bc_mid(ap2d, nch, width=F):
                # [128, nch] -> [128, nch, width] (inner bcast)
                return ap2d.rearrange("p (c o) -> p c o", o=1).to_broadcast(
                    [P, nch, width]
                )

            def bc_feat(ap2d, nch):
                # [128, F] -> [128, nch, F] (middle bcast)
                return ap2d.rearrange("p (o f) -> p o f", o=1).to_broadcast(
                    [P, nch, F]
                )

            with tc.tile_pool(name="ps", bufs=2, space="PSUM") as ps:

                def transform(kind, in_buf, n_chunks, Wb, dis, hs_tag):
                    """hs = dis * (h @ W); bf16 PE path. in_buf: bf16 xT for
                    kind='x', else f32 node-major h (cast to bf16 here)."""
                    hs = pp.tile([P, n_chunks * F], FP32, tag=hs_tag)
                    if kind == "x":
                        inb = in_buf
                    else:
                        inb = pp.tile([P, n_chunks * F], BF16, tag=hs_tag + "_inb")
                        nc.scalar.copy(out=inb[:], in_=in_buf[:, : n_chunks * F])
                    for g0 in range(0, n_chunks, 4):
                        nch = min(4, n_chunks - g0)
                        if kind == "x":
                            zT_ps = ps.tile([F, 512], FP32, tag="zT", space="PSUM")
                            nc.tensor.matmul(
                                out=zT_ps[:, : nch * P],
                                lhsT=Wb[:],
                                rhs=inb[:, g0 * P : (g0 + nch) * P],
                                start=True, stop=True,
                            )
                        else:
                            hT_ps = ps.tile([F, 512], BF16, tag="hT", space="PSUM")
                            for k in range(nch):
                                nc.tensor.transpose(
                                    out=hT_ps[:, k * P : (k + 1) * P],
                                    in_=inb[:, (g0 + k) * F : (g0 + k + 1) * F],
                                    identity=identb[:],
                                )
                            hT_sb = tp.tile([F, 512], BF16, tag="hT_sb")
                            nc.scalar.copy(
                                out=hT_sb[:, : nch * P], in_=hT_ps[:, : nch * P]
                            )
                            zT_ps = ps.tile([F, 512], FP32, tag="zT", space="PSUM")
                            nc.tensor.matmul(
                                out=zT_ps[:, : nch * P],
                                lhsT=Wb[:],
                                rhs=hT_sb[:, : nch * P],
                                start=True, stop=True,
                            )
                        zT_sb = tp.tile([F, 512], BF16, tag="zT_sb")
                        nc.scalar.copy(out=zT_sb[:, : nch * P], in_=zT_ps[:, : nch * P])
                        zN_ps = ps.tile([P, 4 * F], BF16, tag="zN", space="PSUM")
                        for k in range(nch):
                            nc.tensor.transpose(
                                out=zN_ps[:, k * F : (k + 1) * F],
                                in_=zT_sb[:, k * P : (k + 1) * P],
                                identity=identb[:F, :F],
                            )
                        nc.vector.tensor_tensor(
                            out=hs[:, g0 * F : (g0 + nch) * F].rearrange(
                                "p (c f) -> p c f", c=nch
                            ),
                            in0=zN_ps[:, : nch * F].rearrange("p (c f) -> p c f", c=nch),
                            in1=bc_mid(dis[:, g0 : g0 + nch], nch),
                            op=ALU.mult,
                        )
                    return hs

                def broadcast(gp, hs, hsb_tag, stripe, table, n_chunks, rows, dt8):
                    """write own stripe (casting in SBUF), AllGather."""
                    if dt8:
                        hs_lo = gp.tile([P, n_chunks * F], FP8, tag=hsb_tag)
                        nc.scalar.copy(out=hs_lo[:], in_=hs[:, : n_chunks * F])
                        nc.sync.dma_start(
                            out=stripe[0 : n_chunks * P, :].rearrange(
                                "(p c) f -> p (c f)", p=P
                            ),
                            in_=hs_lo[:],
                        )
                    else:
                        nc.gpsimd.dma_start(
                            out=stripe[0 : n_chunks * P, :].rearrange(
                                "(p c) f -> p (c f)", p=P
                            ),
                            in_=hs[:, : n_chunks * F],
                        )
                    if "collectives" in ABLATE:
                        return None
                    nc.gpsimd.collective_compute(
                        "AllGather",
                        ALU.bypass,
                        replica_groups=[list(range(NC))],
                        ins=[stripe[0 : n_chunks * P, :]],
                        outs=[table[0:rows, :]],
                    )
                    return None

                def self_init(gp, A, stripe, sli, n_chunks, tag, gdt=BF16):
                    """A = own rows gathered from local stripe (pre-AllGather)."""
                    sg = gp.tile([P, n_chunks * F], gdt, tag=tag)
                    for ch in range(n_chunks):
                        nc.gpsimd.indirect_dma_start(
                            out=sg[:, ch * F : (ch + 1) * F],
                            out_offset=None,
                            in_=stripe[:],
                            in_offset=bass.IndirectOffsetOnAxis(
                                ap=sli[:, ch : ch + 1], axis=0
                            ),
                        )
                    nc.scalar.copy(out=A[:], in_=sg[:])

                def aggregate(gp, A, table, idxt, cols_list, ni, gtag, gdt=BF16):
                    if "gathers" in ABLATE:
                        return
                    g = gp.tile([P, ni * F], gdt, tag=gtag)
                    for c in range(ni):
                        nc.gpsimd.indirect_dma_start(
                            out=g[:, c * F : (c + 1) * F],
                            out_offset=None,
                            in_=table[:],
                            in_offset=bass.IndirectOffsetOnAxis(
                                ap=idxt[:, c : c + 1], axis=0
                            ),
                        )

                    off = 0
                    for cols in cols_list:
                        nc.vector.tensor_add(
                            out=A[:, : cols * F],
                            in0=A[:, : cols * F],
                            in1=g[:, off * F : (off + cols) * F],
                        )
                        off += cols

                def finish(A, dis, b, n_chunks, h_tag):
                    A3d = A[:].rearrange("p (c f) -> p c f", c=n_chunks)
                    nc.vector.tensor_tensor(
                        out=A3d, in0=A3d, in1=bc_mid(dis[:], n_chunks), op=ALU.mult
                    )
                    nc.vector.tensor_tensor(
                        out=A3d, in0=A3d, in1=bc_feat(b[:], n_chunks), op=ALU.add
                    )
                    h = pp.tile([P, n_chunks * F], FP32, tag=h_tag)
                    nc.scalar.activation(out=h[:], in_=A[:], func=AF.Tanh)
                    return h

                # ---- layer 1 ----
                with tc.tile_pool(name="xp", bufs=1) as xp:
                    xT8 = xp.tile([P, SP1], FP8, tag="xT8")
                    xT = xp.tile([P, SP1], BF16, tag="xT")
                    # chunked load+upcast so transform pipelines with the DMA
                    NL = 4
                    step = ((CH1 + NL - 1) // NL) * P
                    for c0 in range(0, SP1, step):
                        c1 = min(SP1, c0 + step)
                        nc.sync.dma_start(out=xT8[:, c0:c1], in_=xT_e[:, c0:c1])
                        nc.scalar.copy(out=xT[:, c0:c1], in_=xT8[:, c0:c1])
                    hs1 = transform("x", xT, CH1, W1b, dis1, "hs1")
                with tc.tile_pool(name="g1p", bufs=1) as g1p:
                    broadcast(g1p, hs1, "hs1b", stripe1, table1, CH1, NC * SP1, True)
                    aggregate(g1p, hs1, table1, idx1, meta["cols1"], NI1, "G1", FP8)

                    # ---- layer 1 finish + layer 2 transform, staggered per
                    # 4-chunk group: high-index chunks have low in-degree and
                    # complete after the first few gather rounds, so their
                    # finish/transform2/stripe2-write overlaps the remaining
                    # layer-1 gathers, letting AllGather-2 start earlier.
                    h1 = pp.tile([P, CH1 * F], FP32, tag="h1")
                    h1b = pp.tile([P, CH1 * F], BF16, tag="h1b")
                    hs2 = pp.tile([P, CH1 * F], FP32, tag="hs2")
                    hs2_lo = pp.tile([P, CH1 * F], FP8, tag="hs2_lo")
                    for g0 in reversed(range(0, CH1, 4)):
                        nch = min(4, CH1 - g0)
                        sl = slice(g0 * F, (g0 + nch) * F)
                        A3d = hs1[:, sl].rearrange("p (c f) -> p c f", c=nch)
                        nc.vector.tensor_tensor(
                            out=A3d, in0=A3d,
                            in1=bc_mid(dis1[:, g0 : g0 + nch], nch), op=ALU.mult,
                        )
                        nc.vector.tensor_tensor(
                            out=A3d, in0=A3d, in1=bc_feat(b1[:], nch), op=ALU.add,
                        )
                        nc.scalar.activation(
                            out=h1[:, sl], in_=hs1[:, sl], func=AF.Tanh
                        )
                        nc.scalar.copy(out=h1b[:, sl], in_=h1[:, sl])
                        # transform2 on this group: hs2 = dis1 * (h1 @ W2)
                        hT_ps = ps.tile([F, 512], BF16, tag="hT", space="PSUM")
                        for k in range(nch):
                            nc.tensor.transpose(
                                out=hT_ps[:, k * P : (k + 1) * P],
                                in_=h1b[:, (g0 + k) * F : (g0 + k + 1) * F],
                                identity=identb[:],
                            )
                        hT_sb = tp.tile([F, 512], BF16, tag="hT_sb")
                        nc.scalar.copy(
                            out=hT_sb[:, : nch * P], in_=hT_ps[:, : nch * P]
                        )
                        zT_ps = ps.tile([F, 512], FP32, tag="zT", space="PSUM")
                        nc.tensor.matmul(
                            out=zT_ps[:, : nch * P], lhsT=W2b[:],
                            rhs=hT_sb[:, : nch * P], start=True, stop=True,
                        )
                        zT_sb = tp.tile([F, 512], BF16, tag="zT_sb")
                        nc.scalar.copy(out=zT_sb[:, : nch * P], in_=zT_ps[:, : nch * P])
                        zN_ps = ps.tile([P, 4 * F], BF16, tag="zN", space="PSUM")
                        for k in range(nch):
                            nc.tensor.transpose(
                                out=zN_ps[:, k * F : (k + 1) * F],
                                in_=zT_sb[:, k * P : (k + 1) * P],
                                identity=identb[:F, :F],
                            )
                        nc.vector.tensor_tensor(
                            out=hs2[:, sl].rearrange("p (c f) -> p c f", c=nch),
                            in0=zN_ps[:, : nch * F].rearrange("p (c f) -> p c f", c=nch),
                            in1=bc_mid(dis1[:, g0 : g0 + nch], nch),
                            op=ALU.mult,
                        )
                        nc.scalar.copy(out=hs2_lo[:, sl], in_=hs2[:, sl])
                        nc.sync.dma_start(
                            out=stripe2[0 : CH1 * P, :].rearrange(
                                "(p c) f -> p (c f)", p=P
                            )[:, sl],
                            in_=hs2_lo[:, sl],
                        )
                nc.sync.dma_start(
                    out=h1_d[:].rearrange("(p c) f -> p (c f)", p=P), in_=h1[:]
                )
                # readout row-gathers emitted early so they overlap the
                # layer-2/3 gather streams on the Pool queue
                cat = pp.tile([P, 96], FP32, tag="cat")
                nc.gpsimd.indirect_dma_start(
                    out=cat[:, 0:F], out_offset=None, in_=h1_d[:],
                    in_offset=bass.IndirectOffsetOnAxis(ap=r1i[:, 0:1], axis=0),
                )

                # ---- layer 2 ----
                A2 = pp.tile([P, CH2 * F], FP32, tag="A2")
                with tc.tile_pool(name="g2p", bufs=1) as g2p:
                    if "collectives" not in ABLATE:
                        nc.gpsimd.collective_compute(
                            "AllGather",
                            ALU.bypass,
                            replica_groups=[list(range(NC))],
                            ins=[stripe2[0 : CH1 * P, :]],
                            outs=[table2[0 : NC * SP1, :]],
                        )
                    self_init(g2p, A2, stripe2, sl2i, CH2, "sg2", FP8)
                    aggregate(g2p, A2, table2, idx2, meta["cols2"], NI2, "G2", FP8)

                    # layer-2 finish + layer-3 transform, staggered per group
                    # (same trick as layer 1->2; CH2=16 so 4 groups)
                    h2 = pp.tile([P, CH2 * F], FP32, tag="h2")
                    h2b = pp.tile([P, CH2 * F], BF16, tag="h2b")
                    hs3 = pp.tile([P, CH2 * F], FP32, tag="hs3")
                    hs3_lo = pp.tile([P, CH2 * F], BF16, tag="hs3_lo")
                    for g0 in reversed(range(0, CH2, 4)):
                        nch = min(4, CH2 - g0)
                        sl = slice(g0 * F, (g0 + nch) * F)
                        A3d = A2[:, sl].rearrange("p (c f) -> p c f", c=nch)
                        nc.vector.tensor_tensor(
                            out=A3d, in0=A3d,
                            in1=bc_mid(dis2[:, g0 : g0 + nch], nch), op=ALU.mult,
                        )
                        nc.vector.tensor_tensor(
                            out=A3d, in0=A3d, in1=bc_feat(b2[:], nch), op=ALU.add,
                        )
                        nc.scalar.activation(
                            out=h2[:, sl], in_=A2[:, sl], func=AF.Tanh
                        )
                        nc.scalar.copy(out=h2b[:, sl], in_=h2[:, sl])
                        hT_ps = ps.tile([F, 512], BF16, tag="hT", space="PSUM")
                        for k in range(nch):
                            nc.tensor.transpose(
                                out=hT_ps[:, k * P : (k + 1) * P],
                                in_=h2b[:, (g0 + k) * F : (g0 + k + 1) * F],
                                identity=identb[:],
                            )
                        hT_sb = tp.tile([F, 512], BF16, tag="hT_sb")
                        nc.scalar.copy(
                            out=hT_sb[:, : nch * P], in_=hT_ps[:, : nch * P]
                        )
                        zT_ps = ps.tile([F, 512], FP32, tag="zT", space="PSUM")
                        nc.tensor.matmul(
                            out=zT_ps[:, : nch * P], lhsT=W3b[:],
                            rhs=hT_sb[:, : nch * P], start=True, stop=True,
                        )
                        zT_sb = tp.tile([F, 512], BF16, tag="zT_sb")
                        nc.scalar.copy(out=zT_sb[:, : nch * P], in_=zT_ps[:, : nch * P])
                        zN_ps = ps.tile([P, 4 * F], BF16, tag="zN", space="PSUM")
                        for k in range(nch):
                            nc.tensor.transpose(
                                out=zN_ps[:, k * F : (k + 1) * F],
                                in_=zT_sb[:, k * P : (k + 1) * P],
                                identity=identb[:F, :F],
                            )
                        nc.vector.tensor_tensor(
                            out=hs3[:, sl].rearrange("p (c f) -> p c f", c=nch),
                            in0=zN_ps[:, : nch * F].rearrange("p (c f) -> p c f", c=nch),
                            in1=bc_mid(dis2[:, g0 : g0 + nch], nch),
                            op=ALU.mult,
                        )
                        nc.scalar.copy(out=hs3_lo[:, sl], in_=hs3[:, sl])
                        nc.sync.dma_start(
                            out=stripe3[0 : CH2 * P, :].rearrange(
                                "(p c) f -> p (c f)", p=P
                            )[:, sl],
                            in_=hs3_lo[:, sl],
                        )
                nc.sync.dma_start(
                    out=h2_d[:].rearrange("(p c) f -> p (c f)", p=P), in_=h2[:]
                )
                nc.gpsimd.indirect_dma_start(
                    out=cat[:, F : 2 * F], out_offset=None, in_=h2_d[:],
                    in_offset=bass.IndirectOffsetOnAxis(ap=r2i[:, 0:1], axis=0),
                )

                # ---- layer 3 ----
                A3 = pp.tile([P, CH3 * F], FP32, tag="A3")
                with tc.tile_pool(name="g3p", bufs=1) as g3p:
                    if "collectives" not in ABLATE:
                        nc.gpsimd.collective_compute(
                            "AllGather",
                            ALU.bypass,
                            replica_groups=[list(range(NC))],
                            ins=[stripe3[0 : CH2 * P, :]],
                            outs=[table3[0 : NC * SP2, :]],
                        )
                    self_init(g3p, A3, stripe3, sl3i, CH3, "sg3")
                    aggregate(g3p, A3, table3, idx3, meta["cols3"], NI3, "G3")
                    h3 = finish(A3, dis3, b3, CH3, "h3")

            # ---- readout ----
            with (
                tc.tile_pool(name="rp", bufs=1, space="PSUM") as rp,
                tc.tile_pool(name="rsb", bufs=1) as rsb,
            ):
                nc.vector.tensor_copy(out=cat[:, 2 * F : 3 * F], in_=h3[:, :F])

                cT_ps = rp.tile([96, P], FP32, tag="cT", space="PSUM")
                nc.tensor.transpose(out=cT_ps[:], in_=cat[:], identity=ident[:])
                cT = rsb.tile([96, P], FP32, tag="cTs")
                nc.scalar.copy(out=cT[:], in_=cT_ps[:])
                hid_ps = rp.tile([P, P], FP32, tag="hid", space="PSUM")
                nc.tensor.matmul(out=hid_ps[:], lhsT=cT[:], rhs=l1w[:], start=True, stop=True)
                hid = rsb.tile([P, P], FP32, tag="hids")
                nc.vector.tensor_add(out=hid[:], in0=hid_ps[:], in1=l1b[:])
                hidr = rsb.tile([P, P], FP32, tag="hidr")
                nc.scalar.activation(out=hidr[:], in_=hid[:], func=AF.Relu)
                hT_ps = rp.tile([P, P], FP32, tag="hT2", space="PSUM")
                nc.tensor.transpose(out=hT_ps[:], in_=hidr[:], identity=ident[:])
                hT = rsb.tile([P, P], FP32, tag="hT2s")
                nc.scalar.copy(out=hT[:], in_=hT_ps[:])
                lg_ps = rp.tile([P, 2], FP32, tag="lg", space="PSUM")
                nc.tensor.matmul(out=lg_ps[:], lhsT=hT[:], rhs=l2w[:], start=True, stop=True)
                lg = rsb.tile([P, 2], FP32, tag="lgs")
                nc.vector.tensor_add(out=lg[:], in0=lg_ps[:], in1=l2b[:])
                m = rsb.tile([P, 1], FP32, tag="m")
                nc.vector.tensor_reduce(out=m[:], in_=lg[:], axis=mybir.AxisListType.X, op=ALU.max)
                t = rsb.tile([P, 2], FP32, tag="t")
                nc.vector.tensor_scalar(out=t[:], in0=lg[:], scalar1=m[:], scalar2=None, op0=ALU.subtract)
                e = rsb.tile([P, 2], FP32, tag="e")
                nc.scalar.activation(out=e[:], in_=t[:], func=AF.Exp)
                s = rsb.tile([P, 1], FP32, tag="s")
                nc.vector.tensor_reduce(out=s[:], in_=e[:], axis=mybir.AxisListType.X, op=ALU.add)
                ls = rsb.tile([P, 1], FP32, tag="ls")
                nc.scalar.activation(out=ls[:], in_=s[:], func=AF.Ln)
                o = rsb.tile([P, 2], FP32, tag="o")
                nc.vector.tensor_scalar(out=o[:], in0=t[:], scalar1=ls[:], scalar2=None, op0=ALU.subtract)
                nc.sync.dma_start(out=out_e[:], in_=o[:])

    _split_waits(nc)
    return nc


# ---------------------------------------------------------------------------
# entry point
# ---------------------------------------------------------------------------

_CACHE = {}


def _get_runner(meta, consts):
    import hashlib

    h = hashlib.sha1()
    for name in ("idxcat", "degcat", "slcat", "iota_f"):
        h.update(consts[name].tobytes())
    key = (
        meta["CH1"], meta["CH2"], meta["CH3"], meta["NI1"], meta["NI2"], meta["NI3"],
        tuple(meta["cols1"]), tuple(meta["cols2"]), tuple(meta["cols3"]),
        h.hexdigest(),
    )
    if key not in _CACHE:
        nc = _build(meta, consts)
        _CACHE[key] = _SpmdRunner(nc)
    return _CACHE[key]


def kernel(x, edge_index, batch, num_graphs,
           W1, b1, W2, b2, W3, b3, lin1_w, lin1_b, lin2_w, lin2_b):
    x = np.asarray(x, np.float32)
    edge_index = np.asarray(edge_index)
    batch = np.asarray(batch)
    G = int(np.asarray(num_graphs))
    W1 = np.asarray(W1, np.float32)
    W2 = np.asarray(W2, np.float32)
    W3 = np.asarray(W3, np.float32)
    b1 = np.asarray(b1, np.float32)
    b2 = np.asarray(b2, np.float32)
    b3 = np.asarray(b3, np.float32)
    lin1_w = np.asarray(lin1_w, np.float32)
    lin1_b = np.asarray(lin1_b, np.float32)
    lin2_w = np.asarray(lin2_w, np.float32)
    lin2_b = np.asarray(lin2_b, np.float32)

    meta, per_core, slot_graphs = _preprocess(x, edge_index, batch, G)

    # stack per-core tables -> NEFF-baked constants [NC*128, W]
    def stack(key_list, dtype):
        per = [
            np.concatenate([per_core[c][k] for k in key_list], axis=1)
            for c in range(NC)
        ]
        return np.ascontiguousarray(np.stack(per, axis=0).reshape(NC * P, -1), dtype)

    consts = dict(
        idxcat=stack(["idx1", "idx2", "idx3"], np.int32),
        degcat=stack(["deg1", "deg2", "deg3"], np.float32),
        slcat=stack(["sl2", "sl3", "r1", "r2"], np.int32),
        iota_f=np.arange(P, dtype=np.float32).reshape(P, 1),
    )
    runner = _get_runner(meta, consts)

    fp8 = ml_dtypes.float8_e4m3
    in_maps = []
    for c in range(NC):
        pc = per_core[c]
        in_maps.append(
            dict(
                xT=pc["xT"].astype(fp8),
                W1=W1, W2=W2, W3=W3,
                b1=b1[None, :], b2=b2[None, :], b3=b3[None, :],
                l1w=lin1_w.astype(np.float32),
                l1b=lin1_b[None, :],
                l2w=lin2_w.astype(np.float32),
                l2b=lin2_b[None, :],
            )
        )

    args = runner.stage(in_maps)
    outs = runner.run_staged(args)
    res = runner.results(outs)

    logits = np.zeros((G, 2), np.float32)
    for c in range(NC):
        gids = slot_graphs[c]
        logits[gids] = res[c]["out"][: len(gids)]

    # expose for test.py timing
    kernel._last = (runner, args)
    return logits

